# revision 7
# baseline (speedup 1.0000x reference)
"""Trainium2 Bass kernel for CoordLSVotingWeighted (segment_reduce).

Strategy: data-parallel over batch B=8 across 8 NeuronCores (1 image/core).
Host pre-splits `direct` into contiguous dx/dy planes (pure layout, no math).
Per image, on device (engines balanced, 2-way W-chunked pipeline):
  scalar : ew=Exp(w); sp=Ln(ew+1) (softplus, one table switch); sqx=Square(dx);
           hotch = hot*ch (Copy with per-partition scale)
  gpsimd : sqy=dy*dy; pp=dx*dy
  vector : mx=max_c(seg); hot=(seg==mx); hotcw=hot*cw; s=sqx+sqy;
           rinv~=1/s (approx); k=sp*rinv; R00=k*sqy; m=k*pp; R11=k*sqx
  tensor : 32 block-diagonal matmuls psum[96,108] += L[:,g]^T @ R[:,g]
           L = 3 planes [hot|hot*ch|hot*cw] (1024 each),
           R = 3 planes [R00|m|R11] (1152 each); operands are 3D APs.
Host: contract the w-diagonal, assemble 2x2 systems in float64, pinv-solve,
scale by HEIGHT.

Self-contained: only needs numpy / ml_dtypes / concourse (installed env).
"""

import os

import numpy as np

B = 8
H = 128
W = 128
NCLS = 9  # seg channels, class 0 = background
NPTS = 9
OC = 8
HEIGHT = 128.0
N_CORES = 8
NF = W * NPTS  # 1152
KC = 4  # w-columns per matmul group
NG = W // KC  # 32 matmul groups
HW_ = W // 2  # half width (chunking)

_cache: dict = {}


def _build_nc():
    import concourse.bacc as bacc
    import concourse.tile as tile
    import concourse.mybir as mybir
    from concourse.alu_op_type import AluOpType as Alu

    Act = mybir.ActivationFunctionType
    Axis = mybir.AxisListType
    f32 = mybir.dt.float32
    b16 = mybir.dt.bfloat16

    nc = bacc.Bacc(
        "TRN2", target_bir_lowering=False, debug=False, num_devices=N_CORES
    )
    seg_d = nc.dram_tensor("seg", [H, W * NCLS], f32, kind="ExternalInput")
    dx_d = nc.dram_tensor("dx", [H, NF], f32, kind="ExternalInput")
    dy_d = nc.dram_tensor("dy", [H, NF], f32, kind="ExternalInput")
    w_d = nc.dram_tensor("w", [H, NF], f32, kind="ExternalInput")
    cw_d = nc.dram_tensor("cwc8", [H, OC * W], b16, kind="ExternalInput")
    ch_d = nc.dram_tensor("chv", [H, 1], f32, kind="ExternalInput")
    out_d = nc.dram_tensor("acc", [24 * KC, 30 * KC], f32, kind="ExternalOutput")

    # half-slices in the various per-w units
    s9 = [(0, HW_ * NPTS), (HW_ * NPTS, NF)]          # (w p) planes
    s8 = [(0, HW_ * OC), (HW_ * OC, OC * W)]          # (w c) planes
    s1 = [(0, HW_), (HW_, W)]                         # per-w

    with tile.TileContext(nc) as tc:
        with (
            tc.tile_pool(name="main", bufs=1) as pool,
            tc.tile_pool(name="ps", bufs=1, space="PSUM") as psp,
        ):
            # ---- input tiles
            sgt = pool.tile([H, W * NCLS], f32, tag="sgt")
            dxt = pool.tile([H, NF], f32, tag="dxt")
            dyt = pool.tile([H, NF], f32, tag="dyt")
            wdt = pool.tile([H, NF], f32, tag="wdt")
            cwt = pool.tile([H, OC * W], b16, tag="cwt")
            cht = pool.tile([H, 1], f32, tag="cht")

            # DMA order = consumption order; halves pipeline the engines.
            # All on the sync queue (hw DGE), nothing competing early.
            segc = W * NCLS // 2
            nc.sync.dma_start(out=wdt[:, : NF // 2], in_=w_d[:, : NF // 2])
            nc.sync.dma_start(out=wdt[:, NF // 2 :], in_=w_d[:, NF // 2 :])
            nc.sync.dma_start(out=sgt[:, :segc], in_=seg_d[:, :segc])
            nc.sync.dma_start(out=dyt[:, : NF // 2], in_=dy_d[:, : NF // 2])
            nc.sync.dma_start(out=dxt[:, : NF // 2], in_=dx_d[:, : NF // 2])
            nc.sync.dma_start(out=cwt[:, :], in_=cw_d[:, :])
            nc.sync.dma_start(out=sgt[:, segc:], in_=seg_d[:, segc:])
            nc.sync.dma_start(out=dyt[:, NF // 2 :], in_=dy_d[:, NF // 2 :])
            nc.sync.dma_start(out=dxt[:, NF // 2 :], in_=dx_d[:, NF // 2 :])
            nc.sync.dma_start(out=cht[:, :], in_=ch_d[:, :])

            # ---- work tiles (plane-major layouts)
            ew16 = pool.tile([H, NF], b16, tag="ew16")
            sp16 = pool.tile([H, NF], b16, tag="sp16")
            sqx = pool.tile([H, NF], b16, tag="sqx")
            sqy = pool.tile([H, NF], b16, tag="sqy")
            pp = pool.tile([H, NF], b16, tag="pp")
            s32 = pool.tile([H, NF], f32, tag="s32")
            rinv = pool.tile([H, NF], f32, tag="rinv")
            k16 = pool.tile([H, NF], b16, tag="k16")
            mx = pool.tile([H, W], f32, tag="mx")
            # L packed per w: [hot(8) | hot*ch(8) | hot*cw(8)]
            L = pool.tile([H, W * 24], b16, tag="L")
            # R packed per w: [R00(9) pad | R11(9) pad | m(9) pad]
            R = pool.tile([H, W * 30], b16, tag="R")

            sgt_wc = sgt[:, :].rearrange("q (w c) -> q w c", c=NCLS)
            L_w = L[:, :].rearrange("q (w x) -> q w x", x=24)
            R_w = R[:, :].rearrange("q (w f) -> q w f", f=30)
            # zero the pad columns (cols 9, 19, 29 of each w-block)
            nc.vector.memset(R_w[:, :, 9:30:10], 0.0)

            for h in range(2):
                a9, b9_ = s9[h]
                a8, b8_ = s8[h]
                a1, b1_ = s1[h]

                # ---- scalar: softplus chain + dx^2 + hot*ch
                nc.scalar.activation(
                    out=ew16[:, a9:b9_], in_=wdt[:, a9:b9_], func=Act.Exp
                )
                if h == 1:
                    # (table switch to ln happens after both Exps)
                    pass

            for h in range(2):
                a9, b9_ = s9[h]
                nc.scalar.activation(
                    out=sqx[:, a9:b9_], in_=dxt[:, a9:b9_], func=Act.Square
                )
                nc.scalar.activation(
                    out=sp16[:, a9:b9_], in_=ew16[:, a9:b9_], func=Act.Ln,
                    bias=1.0,
                )

            # ---- gpsimd: dy^2 and dx*dy per half
            for h in range(2):
                a9, b9_ = s9[h]
                nc.gpsimd.tensor_tensor(
                    out=sqy[:, a9:b9_], in0=dyt[:, a9:b9_], in1=dyt[:, a9:b9_],
                    op=Alu.mult,
                )
                nc.gpsimd.tensor_tensor(
                    out=pp[:, a9:b9_], in0=dxt[:, a9:b9_], in1=dyt[:, a9:b9_],
                    op=Alu.mult,
                )

            # ---- vector engine, per half
            for h in range(2):
                a9, b9_ = s9[h]
                a8, b8_ = s8[h]
                a1, b1_ = s1[h]
                seg_h = sgt_wc[:, a1:b1_, :]
                nc.vector.tensor_reduce(
                    out=mx[:, a1:b1_], in_=seg_h, axis=Axis.X, op=Alu.max
                )
                mx_b = (
                    mx[:, a1:b1_].unsqueeze(2).broadcast_to((H, HW_, OC))
                )
                hot_h = L_w[:, a1:b1_, 0:8]
                nc.vector.tensor_tensor(
                    out=hot_h, in0=seg_h[:, :, 1:NCLS], in1=mx_b,
                    op=Alu.is_equal,
                )
                cwt_wc = cwt[:, a8:b8_].rearrange("q (w c) -> q w c", c=OC)
                nc.vector.tensor_tensor(
                    out=L_w[:, a1:b1_, 16:24], in0=hot_h,
                    in1=cwt_wc, op=Alu.mult,
                )
                nc.vector.tensor_tensor(
                    out=s32[:, a9:b9_], in0=sqx[:, a9:b9_], in1=sqy[:, a9:b9_],
                    op=Alu.add,
                )
                nc.vector.reciprocal_approx_fast(
                    out=rinv[:, a9:b9_], in_=s32[:, a9:b9_]
                )
                nc.vector.tensor_tensor(
                    out=k16[:, a9:b9_], in0=sp16[:, a9:b9_],
                    in1=rinv[:, a9:b9_], op=Alu.mult,
                )
                k_r = k16[:, a9:b9_].rearrange("q (w p) -> q w p", p=NPTS)
                sqy_r = sqy[:, a9:b9_].rearrange("q (w p) -> q w p", p=NPTS)
                pp_r = pp[:, a9:b9_].rearrange("q (w p) -> q w p", p=NPTS)
                sqx_r = sqx[:, a9:b9_].rearrange("q (w p) -> q w p", p=NPTS)
                nc.vector.tensor_tensor(
                    out=R_w[:, a1:b1_, 0:9], in0=k_r, in1=sqy_r, op=Alu.mult
                )
                nc.vector.tensor_tensor(
                    out=R_w[:, a1:b1_, 10:19], in0=k_r, in1=sqx_r, op=Alu.mult
                )
                nc.vector.tensor_tensor(
                    out=R_w[:, a1:b1_, 20:29], in0=k_r, in1=pp_r, op=Alu.mult
                )
                # scalar: hot*ch for this half
                nc.scalar.mul(L_w[:, a1:b1_, 8:16], hot_h, cht[:, :])

            # ---- segment reduce: 32 block-diagonal accumulating matmuls
            acc = psp.tile([24 * KC, 30 * KC], f32, tag="acc")
            for g in range(NG):
                nc.tensor.matmul(
                    acc[:, :],
                    L[:, g * 24 * KC : (g + 1) * 24 * KC],
                    R[:, g * 30 * KC : (g + 1) * 30 * KC],
                    start=(g == 0),
                    stop=(g == NG - 1),
                )

            outs = pool.tile([24 * KC, 30 * KC], f32, tag="outs")
            nc.scalar.copy(out=outs[:, :], in_=acc[:, :])
            nc.sync.dma_start(out=out_d[:, :], in_=outs[:, :])

    nc.compile()
    return nc


def _host_constants():
    import ml_dtypes

    bf16 = ml_dtypes.bfloat16
    coord = ((np.arange(W, dtype=np.float32) + 0.5) / HEIGHT).astype(bf16)
    cwc8 = np.ascontiguousarray(
        np.broadcast_to(coord[:, None], (W, OC))[None, :, :].repeat(H, axis=0)
    ).reshape(H, W * OC)
    chv = ((np.arange(H, dtype=np.float32) + 0.5) / HEIGHT).reshape(H, 1)
    return cwc8, chv


def _solve_host(acc_f32: np.ndarray) -> np.ndarray:
    """acc [96,120] fp32 -> p [OC, NPTS, 2] fp32 (float64 pinv like ref)."""
    a6 = acc_f32.astype(np.float64).reshape(KC, 3, OC, KC, 30)
    # contract the w-diagonal within each matmul group; cols per w:
    # [R00(9) pad | R11(9) pad | m(9) pad]
    tt = np.einsum("wtcwf->tcf", a6)  # [3, 8, 30]
    A = tt[0, :, 0:9]
    D = tt[0, :, 10:19]
    Bm = tt[0, :, 20:29]
    S1 = tt[1, :, 0:9]
    S3 = tt[1, :, 20:29]
    S2 = tt[2, :, 20:29]
    S4 = tt[2, :, 10:19]
    Rm = np.empty((OC, NPTS, 2, 2), dtype=np.float64)
    Rm[..., 0, 0] = A
    Rm[..., 0, 1] = -Bm
    Rm[..., 1, 0] = -Bm
    Rm[..., 1, 1] = D
    q = np.stack([S1 - S2, S4 - S3], axis=-1)
    Rp = np.linalg.pinv(Rm.reshape(-1, 2, 2)).reshape(Rm.shape)
    p = np.einsum("cpij,cpj->cpi", Rp, q) * HEIGHT
    return p.astype(np.float32)


def kernel(seg, direct, w):
    if "nc" not in _cache:
        _cache["nc"] = _build_nc()
    nc = _cache["nc"]

    seg = np.ascontiguousarray(np.asarray(seg, dtype=np.float32))
    direct = np.asarray(direct, dtype=np.float32)
    w = np.ascontiguousarray(np.asarray(w, dtype=np.float32))
    cwc8, chv = _host_constants()

    d4 = direct.reshape(B, H, W, NPTS, 2)
    dx = np.ascontiguousarray(d4[..., 0]).reshape(B, H, NF)
    dy = np.ascontiguousarray(d4[..., 1]).reshape(B, H, NF)

    in_maps = []
    for i in range(B):
        in_maps.append(
            {
                "seg": seg[i].reshape(H, W * NCLS),
                "dx": dx[i],
                "dy": dy[i],
                "w": w[i].reshape(H, NF),
                "cwc8": cwc8,
                "chv": chv,
            }
        )

    from concourse.bass_utils import run_bass_kernel_spmd

    trace = bool(int(os.environ.get("KERNEL_TRACE", "0")))
    res = run_bass_kernel_spmd(
        nc, in_maps, core_ids=list(range(N_CORES)), trace=trace
    )
    kernel._last_exec_ns = res.exec_time_ns
    kernel._last_results = res

    out = np.stack(
        [_solve_host(np.asarray(res.results[i]["acc"])) for i in range(B)],
        axis=0,
    )
    return out
